# revision 18
# baseline (speedup 1.0000x reference)
"""Trainium2 Bass kernel for nn_MeshFit (retrieval KNN, K=3).

8 NeuronCores, data-parallel over query rows: core i handles class i//2,
query half i%2 (2048 queries x 4096 vertices/features of that class).

Per-core pipeline:
  1. Scores s = -d/2 via ONE bf16 matmul whose 36 K-rows are a manual
     3-way bf16 split of both operands (products exact; score abs err
     ~3e-7 measured, vs ~3e-4 for the stock fp32 matmul path).
  2. Per 2048-column chunk: DVE max8 straight from PSUM; chunk top-8s are
     merged to the row-global top-8, whose indices are recovered with two
     max_index passes (unmatched slots return 0xFFFFFFFF and are min-
     combined; residual garbage slots are healed with the row's top-1).
  3. One gpsimd dma_gather per tile (1024 descriptors, wrapped-16 index
     layout built by a small one-hot "fold" matmul) fetches the 8
     candidates' coords+feats (256B table rows).
  4. Exact refine: d recomputed per candidate in the reference's exact
     fp32 op order, top-3 with reference tie-break (lower index), softmax
     weights, winner-feature gather, weighted sum. Output is bit-exact
     vs the JAX reference on the benchmark data.
"""

import numpy as np
import ml_dtypes

import concourse.bass as bass
import concourse.bacc as bacc
import concourse.mybir as mybir
from concourse.tile import TileContext
from concourse.bass_utils import run_bass_kernel_spmd
from concourse.library_config import mlp

C, N, M, D = 4, 4096, 4096, 32
P = 128
TILES = 16          # query tiles per core (2048 / 128)
KR = 39             # matmul K rows (36 + iota tie-break + 2 pad)
NCORES = 8
QPC = 2048          # queries per core
NCAND = 8           # candidates per query row

_dt = mybir.dt
_BIG = 1.0e6

# master row groups (term-major; rows 3g..3g+2 are coords x,y,z of group g)
#   g:      0        1        2        3        4        5      6..8     9..11
# lhsT:    qh       qh       qh       qm       qm       ql    qsh/m/l   ones
# rhs:     vh       vm       vl       vh       vm       vh      ones   vsh/m/l
_QGROUPS = ["h", "h", "h", "m", "m", "l", "sh", "sm", "sl", "1", "1", "1"]
_VGROUPS = ["h", "m", "l", "h", "m", "h", "1", "1", "1", "sh", "sm", "sl"]


def _emit_split3_into(nc, pool, src32, F, tag, dsts):
    """3-way bf16 split of SBUF fp32 [P, F]; writes pieces h/m/l into the
    given dict of destination APs (bf16)."""
    nc.vector.tensor_copy(dsts["h"], src32[:])
    h32 = pool.tile([P, F], _dt.float32, tag=f"{tag}h32")
    nc.vector.tensor_copy(h32[:], dsts["h"])
    r1 = pool.tile([P, F], _dt.float32, tag=f"{tag}r1")
    nc.vector.tensor_sub(r1[:], src32[:], h32[:])
    nc.vector.tensor_copy(dsts["m"], r1[:])
    m32 = pool.tile([P, F], _dt.float32, tag=f"{tag}m32")
    nc.vector.tensor_copy(m32[:], dsts["m"])
    r2 = pool.tile([P, F], _dt.float32, tag=f"{tag}r2")
    nc.vector.tensor_sub(r2[:], r1[:], m32[:])
    nc.vector.tensor_copy(dsts["l"], r2[:])


def build_nc():
    nc = bacc.Bacc("TRN2", target_bir_lowering=False, debug=False,
                   num_devices=NCORES, dynamic_dma_scratch_size=65536,
                   num_swdge_queues=2)

    qT_d = nc.dram_tensor("qT", [3, QPC], _dt.float32, kind="ExternalInput")
    vT_d = nc.dram_tensor("vT", [3, N], _dt.float32, kind="ExternalInput")
    table_d = nc.dram_tensor("table", [N, 64], _dt.float32, kind="ExternalInput")
    qpt_d = nc.dram_tensor("qpt", [P, TILES * 3], _dt.float32, kind="ExternalInput")
    qg12_d = nc.dram_tensor("qg12", [P, 3 * QPC // P], _dt.bfloat16, kind="ExternalInput")
    vg12_d = nc.dram_tensor("vg12", [P, 3 * N // P], _dt.bfloat16, kind="ExternalInput")
    lfold_d = nc.dram_tensor("lfold", [P, P], _dt.float32, kind="ExternalInput")
    maskj_d = nc.dram_tensor("maskj", [P, P], _dt.float32, kind="ExternalInput")
    out_d = nc.dram_tensor("out", [QPC, D], _dt.float32, kind="ExternalOutput")

    with TileContext(nc) as tc:
        with tc.tile_pool(name="sbuf", bufs=1) as pool, \
             tc.tile_pool(name="prep", bufs=1) as prep, \
             tc.tile_pool(name="psum", bufs=2, space="PSUM") as psum:
            nc.gpsimd.load_library(mlp)

            # ---------- prep: splits in wide layout -> mega tiles ----------
            megas = {}
            for side, nel, src_d, groups in (("q", 3 * QPC, qT_d, _QGROUPS),
                                             ("v", 3 * N, vT_d, _VGROUPS)):
                F = nel // P
                mega = prep.tile([P, 13 * F], _dt.bfloat16, tag=f"{side}mega")
                megas[side] = (mega, F)
                w32 = prep.tile([P, F], _dt.float32, tag=f"{side}w32")
                nc.sync.dma_start(w32[:], src_d[:].rearrange("a b -> (a b)")
                                  .rearrange("(p f) -> p f", p=P))
                gslice = {g: mega[:, g * F:(g + 1) * F] for g in range(12)}
                first = {nm: groups.index(nm) for nm in set(groups)}
                # coordinate pieces
                _emit_split3_into(nc, prep, w32, F, side, {
                    "h": gslice[first["h"]], "m": gslice[first["m"]],
                    "l": gslice[first["l"]]})
                # -0.5 * x^2 pieces
                sq = prep.tile([P, F], _dt.float32, tag=f"{side}sq")
                nc.vector.scalar_tensor_tensor(
                    out=sq[:], in0=w32[:], scalar=-0.5, in1=w32[:],
                    op0=mybir.AluOpType.mult, op1=mybir.AluOpType.mult)
                _emit_split3_into(nc, prep, sq, F, side + "s", {
                    "h": gslice[first["sh"]], "m": gslice[first["sm"]],
                    "l": gslice[first["sl"]]})
                # duplicates + ones
                for g, nm in enumerate(groups):
                    if nm == "1":
                        nc.vector.memset(gslice[g], 1.0)
                    elif g != first[nm]:
                        nc.vector.tensor_copy(gslice[g], gslice[first[nm]])
                # group 12: tie-break row (ones | -iota*2^-35) + 2 zero rows
                g12 = mega[:, 12 * F:13 * F]
                nc.sync.dma_start(g12, qg12_d[:] if side == "q" else vg12_d[:])

            # masters [36, X] bf16: mega -> DRAM staging -> master (2 DMAs/side)
            lhsT = pool.tile([KR, QPC], _dt.bfloat16)
            rhs = pool.tile([KR, N], _dt.bfloat16)
            for (mst, side, X) in ((lhsT, "q", QPC), (rhs, "v", N)):
                mega, F = megas[side]
                stg = nc.dram_tensor(f"stage_{side}", [13 * P * F], _dt.bfloat16)
                nc.sync.dma_start(
                    stg[:].rearrange("(g p f) -> p g f", g=13, p=P),
                    mega[:].rearrange("p (g f) -> p g f", g=13))
                nc.sync.dma_start(
                    mst[:], stg[:].rearrange("(r x) -> r x", r=KR))

            # ---------- consts / collections ----------
            lfold = pool.tile([P, P], _dt.float32)
            nc.sync.dma_start(lfold[:], lfold_d[:])
            maskj = pool.tile([P, P], _dt.float32)
            nc.sync.dma_start(maskj[:], maskj_d[:])
            qpt = pool.tile([P, TILES * 3], _dt.float32)
            nc.sync.dma_start(qpt[:], qpt_d[:])

            NS = TILES * NCAND  # 128 candidate slots
            idx_all = pool.tile([P, NS], _dt.float32)
            wrapped = pool.tile([P, TILES * 64], _dt.int16)
            cand = pool.tile([P, NS, 64], _dt.float32)

            # ---------- exact refine (emitted per half of 8 tiles) ----------
            TH = TILES // 2
            dmat = pool.tile([P, NS], _dt.float32)
            tk = pool.tile([P, NS], _dt.float32, tag="tk")
            t1 = pool.tile([P, NS], _dt.float32, tag="t1")
            dtop = pool.tile([P, TILES * 3], _dt.float32)
            widx = pool.tile([P, TILES * 3], _dt.float32)
            eqm = pool.tile([P, NS], _dt.float32, tag="eqm")
            tmp = pool.tile([P, NS], _dt.float32, tag="tmpr")
            wmask = pool.tile([P, NS], _dt.float32, tag="wmaskr")
            dd = pool.tile([P, TILES * 3], _dt.float32)
            ex = pool.tile([P, TILES * 3], _dt.float32)
            ssum = pool.tile([P, TILES], _dt.float32)
            rec = pool.tile([P, TILES], _dt.float32)
            wgt = pool.tile([P, TILES * 3], _dt.float32)
            wrapped2 = pool.tile([P, TILES * 24], _dt.int16)
            feat2 = pool.tile([P, TILES * 3, 64], _dt.float32)

            def emit_refine(h):
                s0, s1 = h * TH * NCAND, (h + 1) * TH * NCAND
                c0, c1 = h * TH * 3, (h + 1) * TH * 3
                dm = dmat[:, s0:s1]
                ia = idx_all[:, s0:s1]
                for k in range(3):
                    ck = cand[:][:, s0 // 1:s1, k:k + 1]                         .rearrange("p s o -> p (s o)")                         .rearrange("p (t j) -> p t j", t=TH)
                    qk = qpt[:, c0:c1][:, k::3].to_broadcast([P, TH, NCAND])
                    nc.vector.tensor_sub(
                        tk[:, s0:s1].rearrange("p (t j) -> p t j", t=TH), ck, qk)
                    if k == 0:
                        nc.vector.tensor_mul(dm, tk[:, s0:s1], tk[:, s0:s1])
                    else:
                        nc.vector.tensor_mul(t1[:, s0:s1], tk[:, s0:s1], tk[:, s0:s1])
                        nc.vector.tensor_add(dm, dm, t1[:, s0:s1])

                d3 = dm.rearrange("p (t j) -> p t j", t=TH)
                ix3 = ia.rearrange("p (t j) -> p t j", t=TH)
                for r in range(3):
                    dt_r = dtop[:, c0:c1][:, r::3]
                    nc.vector.tensor_reduce(out=dt_r, in_=d3,
                                            op=mybir.AluOpType.min,
                                            axis=mybir.AxisListType.X)
                    nc.vector.tensor_tensor(
                        out=eqm[:, s0:s1].rearrange("p (t j) -> p t j", t=TH),
                        in0=d3, in1=dt_r.to_broadcast([P, TH, NCAND]),
                        op=mybir.AluOpType.is_equal)
                    nc.vector.scalar_tensor_tensor(
                        out=tmp[:, s0:s1], in0=eqm[:, s0:s1], scalar=-_BIG, in1=ia,
                        op0=mybir.AluOpType.mult, op1=mybir.AluOpType.add)
                    wi_r = widx[:, c0:c1][:, r::3]
                    nc.vector.tensor_reduce(
                        out=wi_r,
                        in_=tmp[:, s0:s1].rearrange("p (t j) -> p t j", t=TH),
                        op=mybir.AluOpType.min, axis=mybir.AxisListType.X)
                    nc.vector.tensor_scalar(wi_r, wi_r, _BIG, None,
                                            op0=mybir.AluOpType.add)
                    if r < 2:
                        nc.vector.tensor_tensor(
                            out=wmask[:, s0:s1].rearrange("p (t j) -> p t j", t=TH),
                            in0=ix3, in1=wi_r.to_broadcast([P, TH, NCAND]),
                            op=mybir.AluOpType.is_equal)
                        nc.vector.scalar_tensor_tensor(
                            out=dm, in0=wmask[:, s0:s1], scalar=_BIG, in1=dm,
                            op0=mybir.AluOpType.mult, op1=mybir.AluOpType.add)

                # softmax over winners: w = exp(dmin - d) / sum
                nc.vector.tensor_tensor(
                    out=dd[:, c0:c1].rearrange("p (t r) -> p t r", t=TH),
                    in0=dtop[:, c0:c1].rearrange("p (t r) -> p t r", t=TH),
                    in1=dtop[:, c0:c1][:, 0::3].to_broadcast([P, TH, 3]),
                    op=mybir.AluOpType.subtract)
                nc.scalar.activation(ex[:, c0:c1], dd[:, c0:c1],
                                     mybir.ActivationFunctionType.Exp,
                                     bias=0.0, scale=-1.0)
                nc.vector.tensor_reduce(
                    out=ssum[:, h * TH:(h + 1) * TH],
                    in_=ex[:, c0:c1].rearrange("p (t r) -> p t r", t=TH),
                    op=mybir.AluOpType.add, axis=mybir.AxisListType.X)
                nc.vector.reciprocal(rec[:, h * TH:(h + 1) * TH],
                                     ssum[:, h * TH:(h + 1) * TH])
                nc.vector.tensor_tensor(
                    out=wgt[:, c0:c1].rearrange("p (t r) -> p t r", t=TH),
                    in0=ex[:, c0:c1].rearrange("p (t r) -> p t r", t=TH),
                    in1=rec[:, h * TH:(h + 1) * TH].to_broadcast([P, TH, 3]),
                    op=mybir.AluOpType.mult)

                # winner feature gather for this half
                rmat2 = pool.tile([P, 192], _dt.float32, tag="rmat2")
                nc.vector.tensor_tensor(
                    out=rmat2[:].rearrange("p (s j a) -> p s j a", s=8, j=3),
                    in0=widx[:, c0:c1]
                        .rearrange("p (s j) -> p s j", s=8).to_broadcast([P, 8, 3, 8]),
                    in1=maskj[:, 0:24].rearrange("p (o j a) -> p o j a", o=1, j=3)
                        .broadcast_to([P, 8, 3, 8]),
                    op=mybir.AluOpType.mult)
                fold2_p = psum.tile([P, 192], _dt.float32, space="PSUM", tag="scan")
                nc.tensor.matmul(fold2_p[:], lfold[:], rmat2[:], start=True, stop=True)
                nc.vector.tensor_copy(wrapped2[:, h * 192:(h + 1) * 192], fold2_p[:])
                for gg in range(3):
                    g = h * 3 + gg
                    nc.gpsimd.dma_gather(
                        out_ap=feat2[:, g * 8:(g + 1) * 8, :],
                        in_ap=table_d[:],
                        idxs_ap=wrapped2[:, g * 64:(g + 1) * 64],
                        num_idxs=1024, num_idxs_reg=1024, elem_size=64,
                        queue_num=g % 2)

            # ---------- scan loop ----------
            def emit_fold4(t0):
                # fold 4 tiles' candidate indices at once (one slot steal)
                rmat = pool.tile([P, 256], _dt.float32, tag="rmat")
                nc.vector.tensor_tensor(
                    out=rmat[:].rearrange("p (s j a) -> p s j a", s=4, j=8),
                    in0=idx_all[:, t0 * 8:(t0 + 4) * 8]
                        .rearrange("p (s j) -> p s j", s=4).to_broadcast([P, 4, 8, 8]),
                    in1=maskj[:, 0:64].rearrange("p (o j a) -> p o j a", o=1, j=8)
                        .broadcast_to([P, 4, 8, 8]),
                    op=mybir.AluOpType.mult)
                fold_p = psum.tile([P, 256], _dt.float32, space="PSUM", tag="scan")
                nc.tensor.matmul(fold_p[:], lfold[:], rmat[:], start=True, stop=True)
                nc.vector.tensor_copy(wrapped[:, t0 * 64:(t0 + 4) * 64], fold_p[:])
                for g in range(4):
                    t = t0 + g
                    nc.gpsimd.dma_gather(
                        out_ap=cand[:, t * 8:(t + 1) * 8, :],
                        in_ap=table_d[:],
                        idxs_ap=wrapped[:, t * 64:(t + 1) * 64],
                        num_idxs=1024, num_idxs_reg=1024, elem_size=64,
                        queue_num=g % 2)

            for t in range(TILES):
                vals16 = pool.tile([P, 16], _dt.float32, tag="v16")
                pts = []
                for half in range(2):
                    pt = psum.tile([P, 2048], _dt.float32, space="PSUM", tag="scan")
                    pts.append(pt)
                    for nb in range(4):
                        c0 = half * 2048 + nb * 512
                        nc.tensor.matmul(pt[:, nb * 512:(nb + 1) * 512],
                                         lhsT[:, t * P:(t + 1) * P],
                                         rhs[:, c0:c0 + 512],
                                         start=True, stop=True)
                    nc.vector.max(out=vals16[:, half * 8:(half + 1) * 8], in_=pt[:])
                m8 = pool.tile([P, 8], _dt.float32, tag="m8")
                nc.vector.max(out=m8[:], in_=vals16[:])
                iA = pool.tile([P, 8], _dt.uint32, tag="iA")
                nc.vector.max_index(out=iA[:], in_max=m8[:], in_values=pts[0][:])
                iB = pool.tile([P, 8], _dt.uint32, tag="iB")
                nc.vector.max_index(out=iB[:], in_max=m8[:], in_values=pts[1][:])
                iAf = pool.tile([P, 8], _dt.float32, tag="iAf")
                nc.vector.tensor_scalar(iAf[:], iA[:], 0.0, None,
                                        op0=mybir.AluOpType.add)
                iBf = pool.tile([P, 8], _dt.float32, tag="iBf")
                nc.vector.tensor_scalar(iBf[:], iB[:], 2048.0, None,
                                        op0=mybir.AluOpType.add)
                islot = idx_all[:, t * 8:(t + 1) * 8]
                nc.vector.tensor_tensor(out=islot, in0=iAf[:], in1=iBf[:],
                                        op=mybir.AluOpType.min)
                alt = pool.tile([P, 8], _dt.float32, tag="alt")
                nc.vector.tensor_tensor(out=alt[:], in0=iAf[:], in1=iBf[:],
                                        op=mybir.AluOpType.max)
                # pair-repair: a garbage slot whose value duplicates the
                # previous slot's takes that slot's other-chunk index
                # (value collisions: same score at two columns lands twice
                # in the merged top-8 but matches once per chunk)
                eqp = pool.tile([P, 7], _dt.float32, tag="eqp")
                nc.vector.tensor_tensor(out=eqp[:], in0=m8[:, 1:8],
                                        in1=m8[:, 0:7],
                                        op=mybir.AluOpType.is_equal)
                gz = pool.tile([P, 8], _dt.float32, tag="gz")
                nc.vector.tensor_scalar(gz[:], islot, 4096.0, None,
                                        op0=mybir.AluOpType.is_ge)
                pm = pool.tile([P, 7], _dt.uint32, tag="pm")
                nc.vector.tensor_tensor(out=pm[:], in0=gz[:, 1:8], in1=eqp[:],
                                        op=mybir.AluOpType.mult)
                nc.vector.copy_predicated(islot[:, 1:8], pm[:], alt[:, 0:7])
                # heal any remaining garbage with the row's top-1 index
                gmask = pool.tile([P, 8], _dt.uint32, tag="gmask")
                nc.vector.tensor_scalar(gmask[:], islot, 4096.0, None,
                                        op0=mybir.AluOpType.is_ge)
                nc.vector.copy_predicated(
                    islot, gmask[:],
                    idx_all[:, t * 8:t * 8 + 1].to_broadcast([P, 8]))
                if t % 4 == 3:
                    emit_fold4(t - 3)
                if t == 7:
                    emit_refine(0)
            emit_refine(1)

            # ---------- weighted sum ----------
            # ---------- weighted sum ----------
            acc = pool.tile([P, TILES * D], _dt.float32)
            t2 = pool.tile([P, TILES * D], _dt.float32, tag="t2")

            def f2slice(r):
                return feat2[:][:, r::3, 3:3 + D]

            def wslice(r):
                return wgt[:, r::3].to_broadcast([P, TILES, D])

            a3 = acc[:].rearrange("p (t d) -> p t d", t=TILES)
            t3 = t2[:].rearrange("p (t d) -> p t d", t=TILES)
            nc.vector.tensor_tensor(out=a3, in0=f2slice(0), in1=wslice(0),
                                    op=mybir.AluOpType.mult)
            nc.vector.tensor_tensor(out=t3, in0=f2slice(1), in1=wslice(1),
                                    op=mybir.AluOpType.mult)
            nc.vector.tensor_add(acc[:], acc[:], t2[:])
            nc.vector.tensor_tensor(out=t3, in0=f2slice(2), in1=wslice(2),
                                    op=mybir.AluOpType.mult)
            nc.vector.tensor_add(acc[:], acc[:], t2[:])

            nc.sync.dma_start(
                out_d[:].rearrange("(t p) d -> p t d", p=P),
                acc[:].rearrange("p (t d) -> p t d", t=TILES))

    nc.compile()
    return nc


_NC_CACHE = None


def _get_nc():
    global _NC_CACHE
    if _NC_CACHE is None:
        _NC_CACHE = build_nc()
    return _NC_CACHE


def _consts():
    pidx = np.arange(P)
    lfold = (pidx[:, None] % 16 == pidx[None, :] % 16).astype(np.float32)
    maskj = np.zeros((P, P), np.float32)
    for j in range(16):
        for a in range(8):
            maskj[:, j * 8 + a] = (pidx // 16 == a)
    # tie-break group-12 rows: lhsT side [1;0;0], rhs side [-n*2^-35;0;0]
    qg = np.zeros((3, QPC), np.float32)
    qg[0] = 1.0
    qg12 = qg.reshape(-1).reshape(P, 3 * QPC // P).astype(ml_dtypes.bfloat16)
    vg = np.zeros((3, N), np.float32)
    vg[0] = -(np.arange(N, dtype=np.float64) * 2.0 ** -35)
    vg12 = vg.reshape(-1).reshape(P, 3 * N // P).astype(ml_dtypes.bfloat16)
    return lfold, maskj, qg12, vg12


def _in_maps(points_feat, vertices, new_vertices):
    lfold, maskj, qg12, vg12 = _consts()
    pf = np.ascontiguousarray(np.asarray(points_feat, np.float32))
    V = np.ascontiguousarray(np.asarray(vertices, np.float32))
    Q = np.ascontiguousarray(np.asarray(new_vertices, np.float32))
    in_maps = []
    for core in range(NCORES):
        cls, half = core // 2, core % 2
        q = Q[cls, half * QPC:(half + 1) * QPC]
        table = np.zeros((N, 64), np.float32)
        table[:, 0:3] = V[cls]
        table[:, 3:3 + D] = pf[0, cls * N:(cls + 1) * N]
        qpt = q.reshape(TILES, P, 3).transpose(1, 0, 2).reshape(P, TILES * 3)
        in_maps.append({
            "qT": np.ascontiguousarray(q.T),
            "vT": np.ascontiguousarray(V[cls].T),
            "table": table,
            "qpt": np.ascontiguousarray(qpt),
            "lfold": lfold,
            "maskj": maskj,
            "qg12": qg12,
            "vg12": vg12,
        })
    return in_maps


def kernel(points_feat, vertices, new_vertices):
    nc = _get_nc()
    in_maps = _in_maps(points_feat, vertices, new_vertices)
    res = run_bass_kernel_spmd(nc, in_maps, list(range(NCORES)))
    out = np.empty((1, C * M, D), np.float32)
    for core in range(NCORES):
        cls, half = core // 2, core % 2
        out[0, cls * M + half * QPC: cls * M + (half + 1) * QPC] = \
            res.results[core]["out"]
    return out


# revision 19
# speedup vs baseline: 1.0623x; 1.0623x over previous
"""Trainium2 Bass kernel for nn_MeshFit (retrieval KNN, K=3).

8 NeuronCores, data-parallel over query rows: core i handles class i//2,
query half i%2 (2048 queries x 4096 vertices/features of that class).

Per-core pipeline:
  1. Scores s = -d/2 via ONE bf16 matmul whose 36 K-rows are a manual
     3-way bf16 split of both operands (products exact; score abs err
     ~3e-7 measured, vs ~3e-4 for the stock fp32 matmul path).
  2. Per 2048-column chunk: DVE max8 straight from PSUM; chunk top-8s are
     merged to the row-global top-8, whose indices are recovered with two
     max_index passes (unmatched slots return 0xFFFFFFFF and are min-
     combined; residual garbage slots are healed with the row's top-1).
  3. One gpsimd dma_gather per tile (1024 descriptors, wrapped-16 index
     layout built by a small one-hot "fold" matmul) fetches the 8
     candidates' coords+feats (256B table rows).
  4. Exact refine: d recomputed per candidate in the reference's exact
     fp32 op order, top-3 with reference tie-break (lower index), softmax
     weights, winner-feature gather, weighted sum. Output is bit-exact
     vs the JAX reference on the benchmark data.
"""

import numpy as np
import ml_dtypes

import concourse.bass as bass
import concourse.bacc as bacc
import concourse.mybir as mybir
from concourse.tile import TileContext
from concourse.bass_utils import run_bass_kernel_spmd
from concourse.library_config import mlp

C, N, M, D = 4, 4096, 4096, 32
P = 128
TILES = 16          # query tiles per core (2048 / 128)
KR = 39             # matmul K rows (36 + iota tie-break + 2 pad)
NCORES = 8
QPC = 2048          # queries per core
NCAND = 8           # candidates per query row

_dt = mybir.dt
_BIG = 1.0e6

# master row groups (term-major; rows 3g..3g+2 are coords x,y,z of group g)
#   g:      0        1        2        3        4        5      6..8     9..11
# lhsT:    qh       qh       qh       qm       qm       ql    qsh/m/l   ones
# rhs:     vh       vm       vl       vh       vm       vh      ones   vsh/m/l
_QGROUPS = ["h", "h", "h", "m", "m", "l", "sh", "sm", "sl", "1", "1", "1"]
_VGROUPS = ["h", "m", "l", "h", "m", "h", "1", "1", "1", "sh", "sm", "sl"]


def _emit_split3_into(nc, pool, src32, F, tag, dsts):
    """3-way bf16 split of SBUF fp32 [P, F]; writes pieces h/m/l into the
    given dict of destination APs (bf16)."""
    nc.vector.tensor_copy(dsts["h"], src32[:])
    h32 = pool.tile([P, F], _dt.float32, tag=f"{tag}h32")
    nc.vector.tensor_copy(h32[:], dsts["h"])
    r1 = pool.tile([P, F], _dt.float32, tag=f"{tag}r1")
    nc.vector.tensor_sub(r1[:], src32[:], h32[:])
    nc.vector.tensor_copy(dsts["m"], r1[:])
    m32 = pool.tile([P, F], _dt.float32, tag=f"{tag}m32")
    nc.vector.tensor_copy(m32[:], dsts["m"])
    r2 = pool.tile([P, F], _dt.float32, tag=f"{tag}r2")
    nc.vector.tensor_sub(r2[:], r1[:], m32[:])
    nc.vector.tensor_copy(dsts["l"], r2[:])


def build_nc():
    nc = bacc.Bacc("TRN2", target_bir_lowering=False, debug=False,
                   num_devices=NCORES, dynamic_dma_scratch_size=65536,
                   num_swdge_queues=2)

    qT_d = nc.dram_tensor("qT", [3, QPC], _dt.float32, kind="ExternalInput")
    vT_d = nc.dram_tensor("vT", [3, N], _dt.float32, kind="ExternalInput")
    table_d = nc.dram_tensor("table", [N, 64], _dt.float32, kind="ExternalInput")
    qpt_d = nc.dram_tensor("qpt", [P, TILES * 3], _dt.float32, kind="ExternalInput")
    qg12_d = nc.dram_tensor("qg12", [P, 3 * QPC // P], _dt.bfloat16, kind="ExternalInput")
    vg12_d = nc.dram_tensor("vg12", [P, 3 * N // P], _dt.bfloat16, kind="ExternalInput")
    lfold_d = nc.dram_tensor("lfold", [P, P], _dt.float32, kind="ExternalInput")
    maskj_d = nc.dram_tensor("maskj", [P, P], _dt.float32, kind="ExternalInput")
    out_d = nc.dram_tensor("out", [QPC, D], _dt.float32, kind="ExternalOutput")

    with TileContext(nc) as tc:
        with tc.tile_pool(name="sbuf", bufs=1) as pool, \
             tc.tile_pool(name="prep", bufs=1) as prep, \
             tc.tile_pool(name="psum", bufs=2, space="PSUM") as psum:
            nc.gpsimd.load_library(mlp)

            # ---------- prep: splits in wide layout -> mega tiles ----------
            megas = {}
            for side, nel, src_d, groups in (("q", 3 * QPC, qT_d, _QGROUPS),
                                             ("v", 3 * N, vT_d, _VGROUPS)):
                F = nel // P
                mega = prep.tile([P, 13 * F], _dt.bfloat16, tag=f"{side}mega")
                megas[side] = (mega, F)
                w32 = prep.tile([P, F], _dt.float32, tag=f"{side}w32")
                nc.sync.dma_start(w32[:], src_d[:].rearrange("a b -> (a b)")
                                  .rearrange("(p f) -> p f", p=P))
                gslice = {g: mega[:, g * F:(g + 1) * F] for g in range(12)}
                first = {nm: groups.index(nm) for nm in set(groups)}
                # coordinate pieces
                _emit_split3_into(nc, prep, w32, F, side, {
                    "h": gslice[first["h"]], "m": gslice[first["m"]],
                    "l": gslice[first["l"]]})
                # -0.5 * x^2 pieces
                sq = prep.tile([P, F], _dt.float32, tag=f"{side}sq")
                nc.vector.scalar_tensor_tensor(
                    out=sq[:], in0=w32[:], scalar=-0.5, in1=w32[:],
                    op0=mybir.AluOpType.mult, op1=mybir.AluOpType.mult)
                _emit_split3_into(nc, prep, sq, F, side + "s", {
                    "h": gslice[first["sh"]], "m": gslice[first["sm"]],
                    "l": gslice[first["sl"]]})
                # duplicates + ones
                for g, nm in enumerate(groups):
                    if nm == "1":
                        nc.vector.memset(gslice[g], 1.0)
                    elif g != first[nm]:
                        nc.vector.tensor_copy(gslice[g], gslice[first[nm]])
                # group 12: tie-break row (ones | -iota*2^-35) + 2 zero rows
                g12 = mega[:, 12 * F:13 * F]
                nc.sync.dma_start(g12, qg12_d[:] if side == "q" else vg12_d[:])

            # masters [36, X] bf16: mega -> DRAM staging -> master (2 DMAs/side)
            lhsT = pool.tile([KR, QPC], _dt.bfloat16)
            rhs = pool.tile([KR, N], _dt.bfloat16)
            for (mst, side, X) in ((lhsT, "q", QPC), (rhs, "v", N)):
                mega, F = megas[side]
                stg = nc.dram_tensor(f"stage_{side}", [13 * P * F], _dt.bfloat16)
                nc.sync.dma_start(
                    stg[:].rearrange("(g p f) -> p g f", g=13, p=P),
                    mega[:].rearrange("p (g f) -> p g f", g=13))
                nc.sync.dma_start(
                    mst[:], stg[:].rearrange("(r x) -> r x", r=KR))

            # ---------- consts / collections ----------
            lfold = pool.tile([P, P], _dt.float32)
            nc.sync.dma_start(lfold[:], lfold_d[:])
            maskj = pool.tile([P, P], _dt.float32)
            nc.sync.dma_start(maskj[:], maskj_d[:])
            qpt = pool.tile([P, TILES * 3], _dt.float32)
            nc.sync.dma_start(qpt[:], qpt_d[:])

            NS = TILES * NCAND  # 128 candidate slots
            idx_all = pool.tile([P, NS], _dt.float32)
            wrapped = pool.tile([P, TILES * 64], _dt.int16)
            cand = pool.tile([P, NS, 64], _dt.float32)

            # ---------- exact refine (emitted per half of 8 tiles) ----------
            TH = TILES // 2
            dmat = pool.tile([P, NS], _dt.float32)
            tk = pool.tile([P, NS], _dt.float32, tag="tk")
            t1 = pool.tile([P, NS], _dt.float32, tag="t1")
            dtop = pool.tile([P, TILES * 3], _dt.float32)
            widx = pool.tile([P, TILES * 3], _dt.float32)
            eqm = pool.tile([P, NS], _dt.float32, tag="eqm")
            tmp = pool.tile([P, NS], _dt.float32, tag="tmpr")
            wmask = pool.tile([P, NS], _dt.float32, tag="wmaskr")
            dd = pool.tile([P, TILES * 3], _dt.float32)
            ex = pool.tile([P, TILES * 3], _dt.float32)
            ssum = pool.tile([P, TILES], _dt.float32)
            rec = pool.tile([P, TILES], _dt.float32)
            wgt = pool.tile([P, TILES * 3], _dt.float32)
            wrapped2 = pool.tile([P, TILES * 24], _dt.int16)
            feat2 = pool.tile([P, TILES * 3, 64], _dt.float32)

            def emit_refine(h):
                s0, s1 = h * TH * NCAND, (h + 1) * TH * NCAND
                c0, c1 = h * TH * 3, (h + 1) * TH * 3
                dm = dmat[:, s0:s1]
                ia = idx_all[:, s0:s1]
                for k in range(3):
                    ck = cand[:][:, s0 // 1:s1, k:k + 1]                         .rearrange("p s o -> p (s o)")                         .rearrange("p (t j) -> p t j", t=TH)
                    qk = qpt[:, c0:c1][:, k::3].to_broadcast([P, TH, NCAND])
                    nc.vector.tensor_sub(
                        tk[:, s0:s1].rearrange("p (t j) -> p t j", t=TH), ck, qk)
                    if k == 0:
                        nc.vector.tensor_mul(dm, tk[:, s0:s1], tk[:, s0:s1])
                    else:
                        nc.vector.tensor_mul(t1[:, s0:s1], tk[:, s0:s1], tk[:, s0:s1])
                        nc.vector.tensor_add(dm, dm, t1[:, s0:s1])

                d3 = dm.rearrange("p (t j) -> p t j", t=TH)
                ix3 = ia.rearrange("p (t j) -> p t j", t=TH)
                for r in range(3):
                    dt_r = dtop[:, c0:c1][:, r::3]
                    nc.vector.tensor_reduce(out=dt_r, in_=d3,
                                            op=mybir.AluOpType.min,
                                            axis=mybir.AxisListType.X)
                    nc.vector.tensor_tensor(
                        out=eqm[:, s0:s1].rearrange("p (t j) -> p t j", t=TH),
                        in0=d3, in1=dt_r.to_broadcast([P, TH, NCAND]),
                        op=mybir.AluOpType.is_equal)
                    nc.vector.scalar_tensor_tensor(
                        out=tmp[:, s0:s1], in0=eqm[:, s0:s1], scalar=-_BIG, in1=ia,
                        op0=mybir.AluOpType.mult, op1=mybir.AluOpType.add)
                    wi_r = widx[:, c0:c1][:, r::3]
                    nc.vector.tensor_reduce(
                        out=wi_r,
                        in_=tmp[:, s0:s1].rearrange("p (t j) -> p t j", t=TH),
                        op=mybir.AluOpType.min, axis=mybir.AxisListType.X)
                    nc.vector.tensor_scalar(wi_r, wi_r, _BIG, None,
                                            op0=mybir.AluOpType.add)
                    if r < 2:
                        nc.vector.tensor_tensor(
                            out=wmask[:, s0:s1].rearrange("p (t j) -> p t j", t=TH),
                            in0=ix3, in1=wi_r.to_broadcast([P, TH, NCAND]),
                            op=mybir.AluOpType.is_equal)
                        nc.vector.scalar_tensor_tensor(
                            out=dm, in0=wmask[:, s0:s1], scalar=_BIG, in1=dm,
                            op0=mybir.AluOpType.mult, op1=mybir.AluOpType.add)

                # softmax over winners: w = exp(dmin - d) / sum
                nc.vector.tensor_tensor(
                    out=dd[:, c0:c1].rearrange("p (t r) -> p t r", t=TH),
                    in0=dtop[:, c0:c1].rearrange("p (t r) -> p t r", t=TH),
                    in1=dtop[:, c0:c1][:, 0::3].to_broadcast([P, TH, 3]),
                    op=mybir.AluOpType.subtract)
                nc.scalar.activation(ex[:, c0:c1], dd[:, c0:c1],
                                     mybir.ActivationFunctionType.Exp,
                                     bias=0.0, scale=-1.0)
                nc.vector.tensor_reduce(
                    out=ssum[:, h * TH:(h + 1) * TH],
                    in_=ex[:, c0:c1].rearrange("p (t r) -> p t r", t=TH),
                    op=mybir.AluOpType.add, axis=mybir.AxisListType.X)
                nc.vector.reciprocal(rec[:, h * TH:(h + 1) * TH],
                                     ssum[:, h * TH:(h + 1) * TH])
                nc.vector.tensor_tensor(
                    out=wgt[:, c0:c1].rearrange("p (t r) -> p t r", t=TH),
                    in0=ex[:, c0:c1].rearrange("p (t r) -> p t r", t=TH),
                    in1=rec[:, h * TH:(h + 1) * TH].to_broadcast([P, TH, 3]),
                    op=mybir.AluOpType.mult)

                # winner feature gather for this half
                rmat2 = pool.tile([P, 192], _dt.float32, tag="rmat2")
                nc.vector.tensor_tensor(
                    out=rmat2[:].rearrange("p (s j a) -> p s j a", s=8, j=3),
                    in0=widx[:, c0:c1]
                        .rearrange("p (s j) -> p s j", s=8).to_broadcast([P, 8, 3, 8]),
                    in1=maskj[:, 0:24].rearrange("p (o j a) -> p o j a", o=1, j=3)
                        .broadcast_to([P, 8, 3, 8]),
                    op=mybir.AluOpType.mult)
                fold2_p = psum.tile([P, 192], _dt.float32, space="PSUM", tag="scan")
                nc.tensor.matmul(fold2_p[:], lfold[:], rmat2[:], start=True, stop=True)
                nc.vector.tensor_copy(wrapped2[:, h * 192:(h + 1) * 192], fold2_p[:])
                for gg in range(3):
                    g = h * 3 + gg
                    nc.gpsimd.dma_gather(
                        out_ap=feat2[:, g * 8:(g + 1) * 8, :],
                        in_ap=table_d[:],
                        idxs_ap=wrapped2[:, g * 64:(g + 1) * 64],
                        num_idxs=1024, num_idxs_reg=1024, elem_size=64,
                        queue_num=g % 2)

            # ---------- scan loop ----------
            def emit_fold4(t0):
                # fold 4 tiles' candidate indices at once (one slot steal)
                rmat = pool.tile([P, 256], _dt.float32, tag="rmat")
                nc.vector.tensor_tensor(
                    out=rmat[:].rearrange("p (s j a) -> p s j a", s=4, j=8),
                    in0=idx_all[:, t0 * 8:(t0 + 4) * 8]
                        .rearrange("p (s j) -> p s j", s=4).to_broadcast([P, 4, 8, 8]),
                    in1=maskj[:, 0:64].rearrange("p (o j a) -> p o j a", o=1, j=8)
                        .broadcast_to([P, 4, 8, 8]),
                    op=mybir.AluOpType.mult)
                fold_p = psum.tile([P, 256], _dt.float32, space="PSUM", tag="scan")
                nc.tensor.matmul(fold_p[:], lfold[:], rmat[:], start=True, stop=True)
                nc.vector.tensor_copy(wrapped[:, t0 * 64:(t0 + 4) * 64], fold_p[:])
                for g in range(4):
                    t = t0 + g
                    nc.gpsimd.dma_gather(
                        out_ap=cand[:, t * 8:(t + 1) * 8, :],
                        in_ap=table_d[:],
                        idxs_ap=wrapped[:, t * 64:(t + 1) * 64],
                        num_idxs=1024, num_idxs_reg=1024, elem_size=64,
                        queue_num=g % 2)

            for t in range(TILES):
                vals16 = pool.tile([P, 16], _dt.float32, tag="v16")
                pts = []
                for half in range(2):
                    pt = psum.tile([P, 2048], _dt.float32, space="PSUM", tag="scan")
                    pts.append(pt)
                    for nb in range(4):
                        c0 = half * 2048 + nb * 512
                        nc.tensor.matmul(pt[:, nb * 512:(nb + 1) * 512],
                                         lhsT[:, t * P:(t + 1) * P],
                                         rhs[:, c0:c0 + 512],
                                         start=True, stop=True)
                    nc.vector.max(out=vals16[:, half * 8:(half + 1) * 8], in_=pt[:])
                m8 = pool.tile([P, 8], _dt.float32, tag="m8")
                nc.vector.max(out=m8[:], in_=vals16[:])
                iA = pool.tile([P, 8], _dt.uint32, tag="iA")
                nc.vector.max_index(out=iA[:], in_max=m8[:], in_values=pts[0][:])
                iB = pool.tile([P, 8], _dt.uint32, tag="iB")
                nc.vector.max_index(out=iB[:], in_max=m8[:], in_values=pts[1][:])
                iAf = pool.tile([P, 8], _dt.float32, tag="iAf")
                nc.vector.tensor_scalar(iAf[:], iA[:], 0.0, None,
                                        op0=mybir.AluOpType.add)
                iBf = pool.tile([P, 8], _dt.float32, tag="iBf")
                nc.vector.tensor_scalar(iBf[:], iB[:], 2048.0, None,
                                        op0=mybir.AluOpType.add)
                islot = idx_all[:, t * 8:(t + 1) * 8]
                nc.vector.tensor_tensor(out=islot, in0=iAf[:], in1=iBf[:],
                                        op=mybir.AluOpType.min)
                alt = pool.tile([P, 8], _dt.float32, tag="alt")
                nc.vector.tensor_tensor(out=alt[:], in0=iAf[:], in1=iBf[:],
                                        op=mybir.AluOpType.max)
                # pair-repair: a garbage slot whose value duplicates the
                # previous slot's takes that slot's other-chunk index
                # (value collisions: same score at two columns lands twice
                # in the merged top-8 but matches once per chunk)
                eqp = pool.tile([P, 7], _dt.float32, tag="eqp")
                nc.vector.tensor_tensor(out=eqp[:], in0=m8[:, 1:8],
                                        in1=m8[:, 0:7],
                                        op=mybir.AluOpType.is_equal)
                gz = pool.tile([P, 8], _dt.float32, tag="gz")
                nc.vector.tensor_scalar(gz[:], islot, 4096.0, None,
                                        op0=mybir.AluOpType.is_ge)
                pm = pool.tile([P, 7], _dt.uint32, tag="pm")
                nc.vector.tensor_tensor(out=pm[:], in0=gz[:, 1:8], in1=eqp[:],
                                        op=mybir.AluOpType.mult)
                nc.vector.copy_predicated(islot[:, 1:8], pm[:], alt[:, 0:7])
                # heal any remaining garbage with the row's top-1 index
                gmask = pool.tile([P, 8], _dt.uint32, tag="gmask")
                nc.vector.tensor_scalar(gmask[:], islot, 4096.0, None,
                                        op0=mybir.AluOpType.is_ge)
                nc.vector.copy_predicated(
                    islot, gmask[:],
                    idx_all[:, t * 8:t * 8 + 1].to_broadcast([P, 8]))
                if t % 4 == 3:
                    emit_fold4(t - 3)
            emit_refine(0)
            emit_refine(1)

            # ---------- weighted sum ----------
            # ---------- weighted sum ----------
            acc = pool.tile([P, TILES * D], _dt.float32)
            t2 = pool.tile([P, TILES * D], _dt.float32, tag="t2")

            def f2slice(r):
                return feat2[:][:, r::3, 3:3 + D]

            def wslice(r):
                return wgt[:, r::3].to_broadcast([P, TILES, D])

            a3 = acc[:].rearrange("p (t d) -> p t d", t=TILES)
            t3 = t2[:].rearrange("p (t d) -> p t d", t=TILES)
            nc.vector.tensor_tensor(out=a3, in0=f2slice(0), in1=wslice(0),
                                    op=mybir.AluOpType.mult)
            nc.vector.tensor_tensor(out=t3, in0=f2slice(1), in1=wslice(1),
                                    op=mybir.AluOpType.mult)
            nc.vector.tensor_add(acc[:], acc[:], t2[:])
            nc.vector.tensor_tensor(out=t3, in0=f2slice(2), in1=wslice(2),
                                    op=mybir.AluOpType.mult)
            nc.vector.tensor_add(acc[:], acc[:], t2[:])

            nc.sync.dma_start(
                out_d[:].rearrange("(t p) d -> p t d", p=P),
                acc[:].rearrange("p (t d) -> p t d", t=TILES))

    nc.compile()
    return nc


_NC_CACHE = None


def _get_nc():
    global _NC_CACHE
    if _NC_CACHE is None:
        _NC_CACHE = build_nc()
    return _NC_CACHE


def _consts():
    pidx = np.arange(P)
    lfold = (pidx[:, None] % 16 == pidx[None, :] % 16).astype(np.float32)
    maskj = np.zeros((P, P), np.float32)
    for j in range(16):
        for a in range(8):
            maskj[:, j * 8 + a] = (pidx // 16 == a)
    # tie-break group-12 rows: lhsT side [1;0;0], rhs side [-n*2^-35;0;0]
    qg = np.zeros((3, QPC), np.float32)
    qg[0] = 1.0
    qg12 = qg.reshape(-1).reshape(P, 3 * QPC // P).astype(ml_dtypes.bfloat16)
    vg = np.zeros((3, N), np.float32)
    vg[0] = -(np.arange(N, dtype=np.float64) * 2.0 ** -35)
    vg12 = vg.reshape(-1).reshape(P, 3 * N // P).astype(ml_dtypes.bfloat16)
    return lfold, maskj, qg12, vg12


def _in_maps(points_feat, vertices, new_vertices):
    lfold, maskj, qg12, vg12 = _consts()
    pf = np.ascontiguousarray(np.asarray(points_feat, np.float32))
    V = np.ascontiguousarray(np.asarray(vertices, np.float32))
    Q = np.ascontiguousarray(np.asarray(new_vertices, np.float32))
    in_maps = []
    for core in range(NCORES):
        cls, half = core // 2, core % 2
        q = Q[cls, half * QPC:(half + 1) * QPC]
        table = np.zeros((N, 64), np.float32)
        table[:, 0:3] = V[cls]
        table[:, 3:3 + D] = pf[0, cls * N:(cls + 1) * N]
        qpt = q.reshape(TILES, P, 3).transpose(1, 0, 2).reshape(P, TILES * 3)
        in_maps.append({
            "qT": np.ascontiguousarray(q.T),
            "vT": np.ascontiguousarray(V[cls].T),
            "table": table,
            "qpt": np.ascontiguousarray(qpt),
            "lfold": lfold,
            "maskj": maskj,
            "qg12": qg12,
            "vg12": vg12,
        })
    return in_maps


def kernel(points_feat, vertices, new_vertices):
    nc = _get_nc()
    in_maps = _in_maps(points_feat, vertices, new_vertices)
    res = run_bass_kernel_spmd(nc, in_maps, list(range(NCORES)))
    out = np.empty((1, C * M, D), np.float32)
    for core in range(NCORES):
        cls, half = core // 2, core % 2
        out[0, cls * M + half * QPC: cls * M + (half + 1) * QPC] = \
            res.results[core]["out"]
    return out
